# revision 27
# baseline (speedup 1.0000x reference)
"""Multi-head causal self-attention on 8 Trainium2 NeuronCores (Bass/Tile).

Sharding: heads 2c,2c+1 -> core c (both batches). Each core computes its two
heads' attention output (concat^T rows [128c, 128c+128)), per-batch AllToAlls
redistribute so core c holds concat^T[:, 256c:256c+256] of each batch, and
each core runs the output projection for its two 256-col s-slices over full d.
Host assembles the 8 [1024, 512] slices and transposes back to [2, 2048, 1024].

All matmuls run in fp16 (tolerance 2e-2; measured rel err ~7e-4).

Schedule highlights:
- a tiny dummy AllToAll fires at program start so the collective stack's
  first-use rendezvous cost (~11us trigger delay) is paid off the critical
  path; the real A2As then trigger with ~1us delay;
- one AllToAll per batch; A2A0 triggers right after batch 0's epilogue and
  completes under batch 1's attention;
- exp fused across both heads per chunk via one 3-D strided activation
  (ScalarE is ~95% busy during attention - instruction count matters);
- causal masks on gpsimd for BOTH batches (DVE freed for evict/norm work);
  the cb0 gather runs on the gpsimd SWDGE after the last b1 mask, so it
  never blocks mask progress, and the A2A1 trigger follows it;
- cb1 gather on the sync HWDGE queue after out-proj(b0)'s stores (a DMA
  trigger stalls its issuing engine while waiting, so order matters);
- x tiles stream on both HWDGE queues for the whole projection phase
  (one queue saturates at ~160 GB/s; two cut the phase's DMA stalls);
  wo loads after all x traffic since it's needed only at the end;
- weights host-prearranged to SBUF layout; V evicts split per 128-col
  quarter; output stored fp16 per-chunk; softmax-denom broadcast via cheap
  K=1 fp16 ones-matmuls; PV pipeline depth 2; exp trimmed to valid columns.
"""
import sys

sys.path.insert(0, "/opt/trn_rl_repo")

import numpy as np
import concourse.bacc as bacc
import concourse.mybir as mybir
from concourse.tile import TileContext
from concourse.bass_utils import run_bass_kernel_spmd

B = 2
S = 2048
D = 1024
H = 16
DK = 64
NCORES = 8
HPC = H // NCORES          # heads per core = 2
BS = B * S                 # 4096
NDC = D // 128             # 8 d-chunks
QB = 512                   # q-block width
NQB = S // QB              # 4 q-blocks per batch
NKC = S // 128             # 16 kpos chunks per batch
HS = 256                   # per-core s-slice per batch (A2A shard width)

F32 = mybir.dt.float32
F16 = mybir.dt.float16
AF = mybir.ActivationFunctionType

_COMPILED = {}


def build_program():
    nc = bacc.Bacc(None, target_bir_lowering=False, debug=False)

    # ---- DRAM I/O (per-core tensors; host supplies per-core slices) ----
    xT = nc.dram_tensor("xT", [D, BS], F16, kind="ExternalInput")
    # weights host-prearranged to the exact SBUF layout (contiguous DMAs,
    # no per-256B-descriptor rearrange storms)
    wqT = nc.dram_tensor("wqT", [128, NDC * 128], F16, kind="ExternalInput")
    wkT = nc.dram_tensor("wkT", [128, NDC * 128], F16, kind="ExternalInput")
    wvT = nc.dram_tensor("wvT", [128, NDC * 128], F16, kind="ExternalInput")
    woT = nc.dram_tensor("woT", [128, NDC * D], F16, kind="ExternalInput")
    ident = nc.dram_tensor("ident", [128, 128], F16, kind="ExternalInput")
    tri = nc.dram_tensor("tri", [128, 128], F16, kind="ExternalInput")
    out = nc.dram_tensor("out", [D, 2 * HS], F16, kind="ExternalOutput")
    outr = out.rearrange("(c p) m -> p c m", p=128)

    with TileContext(nc) as tc:
        with (
            tc.tile_pool(name="const", bufs=1) as const,
            tc.tile_pool(name="persist", bufs=1) as persist,
            tc.tile_pool(name="ct", bufs=1) as ctp,
            tc.tile_pool(name="dram", bufs=1, space="DRAM") as dram,
        ):
            a2a_in = [dram.tile([NCORES, 128, HS], F16, name=f"a2ain_{b}")
                      for b in range(B)]
            a2a_out = [dram.tile([NCORES, 128, HS], F16, name=f"a2aout_{b}")
                       for b in range(B)]
            warm_cc = [dram.tile([NCORES, 1, 16], F16, name=f"wcc{i}")
                       for i in range(2)]

            # projection weights first (the first matmul group needs wq);
            # each split across both HWDGE queues to halve load latency
            w_sb = {}
            for name in ("q", "k", "v"):
                w_sb[name] = const.tile([128, NDC * 128], F16, name=f"w{name}")
            nc.sync.dma_start(out=w_sb["q"][:, 0:4 * 128],
                              in_=wqT[:, 0:4 * 128])
            nc.scalar.dma_start(out=w_sb["q"][:, 4 * 128:],
                                in_=wqT[:, 4 * 128:])

            # dummy collective: pays the first-use rendezvous cost early so
            # the real A2As trigger with ~1us delay instead of ~11us
            nc.gpsimd.collective_compute(
                "AllToAll", mybir.AluOpType.bypass,
                replica_groups=[list(range(NCORES))],
                ins=[warm_cc[0].opt()], outs=[warm_cc[1].opt()])

            # small constants (no DMA where possible)
            id_sb = const.tile([128, 128], F16)
            nc.scalar.dma_start(out=id_sb[:], in_=ident[:])
            tri_sb = const.tile([128, 128], F16)
            nc.scalar.dma_start(out=tri_sb[:], in_=tri[:])
            nbias = const.tile([128, 1], F32)
            nc.vector.memset(nbias[:], -6.0)
            # ones row for broadcasting recip-sumexp to 64 partitions via a
            # cheap fp16 K=1 matmul
            ones16 = const.tile([1, 64], F16, name="ones16")
            nc.vector.memset(ones16[:], 1.0)

            # persistent activations
            QT = [persist.tile([128, S], F16, name=f"QT_{b}") for b in range(B)]
            KT = [persist.tile([128, S], F16, name=f"KT_{b}") for b in range(B)]
            # V tiles: [kpos 128, 130] = [Vh0 | 1 | Vh1 | 1] per (b, kpos-chunk)
            VP = [[persist.tile([128, 130], F16, name=f"VP_{b}_{sc}")
                   for sc in range(NKC)] for b in range(B)]
            for b in range(B):
                for sc in range(NKC):
                    nc.vector.memset(VP[b][sc][:, 64:65], 1.0)
                    nc.vector.memset(VP[b][sc][:, 129:130], 1.0)

            # gathered A2A results
            cb_tiles = [ctp.tile([128, NCORES * HS], F16, name=f"cb_{b}",
                                 tag=f"cb{b}") for b in range(B)]

            # ---- phase 1: projections (xt streamed in 512-col tiles) ----
            with (
                tc.tile_pool(name="xt", bufs=24) as xtp,
                tc.tile_pool(name="vt", bufs=2) as vtp,
                tc.tile_pool(name="pacc", bufs=4, space="PSUM") as pacc,
                tc.tile_pool(name="ptr", bufs=2, space="PSUM") as ptr,
            ):
                xt = {}
                for b in range(B):
                    for j in range(4):
                        for dc in range(NDC):
                            t = xtp.tile([128, QB], F16,
                                         name=f"xt_{b}_{j}_{dc}", tag="xt")
                            # both HWDGE queues stream x for the whole phase
                            eng = nc.sync if dc % 2 == 0 else nc.scalar
                            eng.dma_start(
                                out=t[:],
                                in_=xT[dc * 128:(dc + 1) * 128,
                                       b * S + j * QB:b * S + (j + 1) * QB])
                            xt[b, j, dc] = t
                        if b == 0 and j == 0:
                            nc.sync.dma_start(
                                out=w_sb["k"][:, 0:4 * 128],
                                in_=wkT[:, 0:4 * 128])
                            nc.scalar.dma_start(
                                out=w_sb["k"][:, 4 * 128:],
                                in_=wkT[:, 4 * 128:])
                            nc.sync.dma_start(
                                out=w_sb["v"][:, 0:4 * 128],
                                in_=wvT[:, 0:4 * 128])
                            nc.scalar.dma_start(
                                out=w_sb["v"][:, 4 * 128:],
                                in_=wvT[:, 4 * 128:])
                # out-proj weights after all x traffic: one 2MB transfer
                # that is only needed at ~150us
                wo_sb = const.tile([128, NDC * D], F16, name="wo")
                nc.scalar.dma_start(out=wo_sb[:], in_=woT[:])

                proj_scope = nc.named_scope("proj")
                proj_scope.__enter__()

                # warm-up matmuls on the identity tile: ~3us of PE activity so
                # the HAM clock gate un-throttles before the first projection
                # group (otherwise the first ~3.4us of real matmuls run at
                # 1.2GHz)
                warm0 = pacc.tile([128, 128], F32, name="warm0", tag="warm0",
                                  bufs=1)
                for _ in range(10):
                    nc.tensor.matmul(warm0[:], id_sb[:], id_sb[:],
                                     start=True, stop=True)

                for b in range(B):
                    for j in range(4):
                        for mat in ("q", "k", "v"):
                            ps = pacc.tile([128, QB], F32,
                                           name=f"ps_{mat}_{b}_{j}", tag="pacc")
                            for dc in range(NDC):
                                nc.tensor.matmul(
                                    ps[:],
                                    w_sb[mat][:, dc * 128:(dc + 1) * 128],
                                    xt[b, j, dc][:],
                                    start=(dc == 0), stop=(dc == NDC - 1))
                            if mat in ("q", "k"):
                                dst = QT[b] if mat == "q" else KT[b]
                                nc.scalar.copy(dst[:, j * QB:(j + 1) * QB], ps[:])
                            else:
                                vt = vtp.tile([128, QB], F16,
                                              name=f"vt_{b}_{j}", tag="vt")
                                for t4 in range(4):
                                    # evict per 128-col quarter so each
                                    # transpose starts as soon as its slice
                                    # lands instead of after the full copy
                                    nc.scalar.copy(
                                        vt[:, t4 * 128:(t4 + 1) * 128],
                                        ps[:, t4 * 128:(t4 + 1) * 128])
                                for t4 in range(4):
                                    sc = 4 * j + t4
                                    tr = ptr.tile([128, 128], F16,
                                                  name=f"tr_{b}_{sc}", tag="tr")
                                    nc.tensor.transpose(
                                        tr[:], vt[:, t4 * 128:(t4 + 1) * 128],
                                        id_sb[:])
                                    nc.vector.tensor_copy(
                                        VP[b][sc][:, 0:64], tr[:, 0:64])
                                    nc.vector.tensor_copy(
                                        VP[b][sc][:, 65:129], tr[:, 64:128])

            proj_scope.__exit__(None, None, None)
            # ---- phase 2: attention + per-batch A2A ----
            with (
                tc.tile_pool(name="pp", bufs=8) as pp,
                tc.tile_pool(name="osb", bufs=2) as osb,
            ):
                attn_psum = [
                    tc.tile_pool(name="psc", bufs=2, space="PSUM"),
                    tc.tile_pool(name="po", bufs=3, space="PSUM"),
                    tc.tile_pool(name="pbc", bufs=1, space="PSUM"),
                ]
                psc, po, pbc = [p.__enter__() for p in attn_psum]

                attn_scope = nc.named_scope("attn")
                attn_scope.__enter__()

                # Flat chunk stream with a cross-block 2-deep PV pipeline.
                # All of b0 first: A2A0 dispatches at mid-attention and its
                # completion hides entirely under batch 1's compute.
                order = [(0, 0), (0, 1), (0, 2), (0, 3),
                         "A2A0", (1, 0), (1, 1), (1, 2), (1, 3), "A2A1"]

                class Blk:
                    pass

                stream = []
                for item in order:
                    if isinstance(item, str):
                        stream.append(item)
                        continue
                    b, jb = item
                    blk = Blk()
                    blk.b, blk.jb = b, jb
                    blk.nchunk = 4 * jb + 4
                    stream.extend((blk, i) for i in range(blk.nchunk))

                def emit_a2a(b):
                    nc.gpsimd.collective_compute(
                        "AllToAll",
                        mybir.AluOpType.bypass,
                        replica_groups=[list(range(NCORES))],
                        ins=[a2a_in[b].opt()],
                        outs=[a2a_out[b].opt()],
                    )

                def emit_pv(blk, i, v0):
                    for h in range(HPC):
                        nc.tensor.matmul(
                            blk.o_ps[h][:, v0:QB],
                            VP[blk.b][i][:, 65 * h:65 * h + 65],
                            blk.pblk[i][:, h:h + 1, v0:QB],
                            start=(i == 0), stop=(i == blk.nchunk - 1))
                    if i == blk.nchunk - 1:
                        emit_evict(blk)

                def emit_evict(blk):
                    # evict accumulators (frees po PSUM) + fast reciprocal of
                    # both heads' sumexp rows (read straight from PSUM)
                    b, jb = blk.b, blk.jb
                    blk.ou = [osb.tile([65, QB], F32, name=f"ou_{b}_{jb}_{h}",
                                       tag="ou") for h in range(HPC)]
                    se = osb.tile([1, 2 * QB], F32, name=f"se_{b}_{jb}",
                                  tag="se")
                    for h in range(HPC):
                        nc.vector.tensor_copy(blk.ou[h][:], blk.o_ps[h][:])
                        nc.vector.tensor_copy(
                            se[0:1, h * QB:(h + 1) * QB],
                            blk.o_ps[h][64:65, :])
                    rc = osb.tile([1, 2 * QB], F32, name=f"rx_{b}_{jb}",
                                  tag="rc2")
                    nc.vector.reciprocal_approx_fast(rc[:], se[:])
                    rr = osb.tile([1, 2 * QB], F16, name=f"rr_{b}_{jb}",
                                  tag="recr")
                    nc.vector.tensor_copy(rr[:], rc[:])
                    blk.rec_r = rr
                    pending_norm.append(blk)

                def flush_norm():
                    while pending_norm:
                        blk = pending_norm.pop(0)
                        b, jb = blk.b, blk.jb
                        o2 = osb.tile([128, QB], F16, name=f"o2_{b}_{jb}",
                                      tag="o2")
                        bc = pbc.tile([128, QB], F32, name=f"bc_{b}_{jb}",
                                      tag="bc")
                        for h in range(HPC):
                            nc.tensor.matmul(
                                bc[h * 64:(h + 1) * 64, :], ones16[:],
                                blk.rec_r[0:1, h * QB:(h + 1) * QB],
                                start=True, stop=True)
                        for h in range(HPC):
                            nc.vector.tensor_tensor(
                                out=o2[h * 64:(h + 1) * 64, :],
                                in0=blk.ou[h][0:64, :],
                                in1=bc[h * 64:(h + 1) * 64, :],
                                op=mybir.AluOpType.mult)
                        for half in range(2):
                            nc.sync.dma_start(
                                out=a2a_in[b][2 * jb + half, :, :],
                                in_=o2[:, half * HS:(half + 1) * HS])

                pending_norm = []
                pvq = []           # [(blk, i, v0)] awaiting emission (depth 2)
                nchunks_seen = 0
                for item in stream:
                    if isinstance(item, str):
                        # flush pipeline + norms before the collective so its
                        # input DMAs are emitted
                        while pvq:
                            emit_pv(*pvq.pop(0))
                        flush_norm()
                        if item == "A2A0":
                            emit_a2a(0)
                        else:
                            # trigger A2A1 BEFORE the cb0 gather: the gather
                            # blocks gpsimd until A2A0 completes, and the
                            # trigger must not sit behind that wait (it only
                            # needs b1's epilogue stores)
                            emit_a2a(1)
                            nc.gpsimd.dma_start(
                                out=cb_tiles[0][:],
                                in_=a2a_out[0].rearrange("r p m -> p r m"))
                        continue
                    blk, i = item
                    b, jb = blk.b, blk.jb
                    if i == 0:
                        blk.o_ps = [po.tile([65, QB], F32,
                                            name=f"o_{b}_{jb}_{h}", tag="po")
                                    for h in range(HPC)]
                        blk.pblk = {}
                    r = i - 4 * jb
                    v0 = max(0, r) * 128   # first valid q col in block
                    s_ps = psc.tile([128, HPC, QB], F32,
                                    name=f"s_{b}_{jb}_{i}", tag="psc")
                    for h in range(HPC):
                        nc.tensor.matmul(
                            s_ps[:, h:h + 1, v0:QB],
                            KT[b][h * 64:(h + 1) * 64, i * 128:(i + 1) * 128],
                            QT[b][h * 64:(h + 1) * 64,
                                  jb * QB + v0:(jb + 1) * QB],
                            start=True, stop=True)
                    nchunks_seen += 1
                    if nchunks_seen % 4 == 2:
                        flush_norm()
                    p_sb = pp.tile([128, HPC, QB], F16,
                                   name=f"p_{b}_{jb}_{i}", tag="pp")
                    blk.pblk[i] = p_sb
                    # bias -6 keeps exp in fp16 range (max raw score*scale is
                    # ~12); normalization cancels it. One strided activation
                    # covers both heads.
                    nc.scalar.activation(p_sb[:, :, v0:QB], s_ps[:, :, v0:QB],
                                         AF.Exp, scale=0.125, bias=nbias[:])
                    if r >= 0:
                        # zero the partial 128-col diagonal block on gpsimd
                        # (idle during attention; collectives don't block it
                        # since the gathers come after all masks)
                        for h in range(HPC):
                            sl = p_sb[:, h:h + 1, v0:v0 + 128]
                            nc.gpsimd.tensor_tensor(
                                out=sl, in0=sl, in1=tri_sb[:],
                                op=mybir.AluOpType.mult)
                    while len(pvq) >= 2:
                        emit_pv(*pvq.pop(0))
                    pvq.append((blk, i, v0))
                while pvq:
                    emit_pv(*pvq.pop(0))
                attn_scope.__exit__(None, None, None)
                for p in reversed(attn_psum):
                    p.__exit__(None, None, None)

                # out-proj(b0) runs as soon as the PE drains attention (cb0
                # gathered under attn-b1's tail); out-proj(b1) after the cb1
                # gather, which queues on sync BEHIND out-proj(b0)'s stores.
                # PSUM pool opened after the attn pools close so the bank
                # allocator reuses their banks.
                with tc.tile_pool(name="pf", bufs=2, space="PSUM") as pf:
                    def warm_fill(n):
                        # dependency-free matmuls bridging residual collective
                        # wait so the HAM clock gate doesn't re-throttle
                        wp = pf.tile([128, QB], F32, name="warm_ps",
                                     tag="warm", bufs=1)
                        for _ in range(n):
                            nc.tensor.matmul(wp[:], id_sb[:], KT[0][:, 0:QB],
                                             start=True, stop=True)

                    def outproj(b):
                        for ds in range(NDC):
                            f_ps = pf.tile([128, HS], F32, name=f"f_{b}_{ds}",
                                           tag="pf")
                            for kc in range(NDC):
                                nc.tensor.matmul(
                                    f_ps[:],
                                    wo_sb[:, kc * D + ds * 128:
                                          kc * D + (ds + 1) * 128],
                                    cb_tiles[b][:, kc * HS:(kc + 1) * HS],
                                    start=(kc == 0), stop=(kc == NDC - 1))
                            fb = osb.tile([128, HS], F16, name=f"fb_{b}_{ds}",
                                          tag="fb")
                            nc.vector.tensor_copy(fb[:], f_ps[:])
                            # alternate queues: halves the final store tail
                            # (the scalar queue is free this late)
                            eng = nc.sync if ds % 2 == 0 else nc.scalar
                            eng.dma_start(
                                out=outr[:, ds, b * HS:(b + 1) * HS],
                                in_=fb[:])

                    warm_fill(8)
                    outproj(0)
                    # cb1 gather on sync AFTER b0's out stores (trigger
                    # stalls the queue head while waiting on the collective)
                    nc.sync.dma_start(
                        out=cb_tiles[1][:],
                        in_=a2a_out[1].rearrange("r p m -> p r m"))
                    warm_fill(8)
                    outproj(1)

    nc.finalize()
    return nc


def prepare_in_maps(x, q_heads, k_heads, v_heads, output_proj):
    x = np.ascontiguousarray(x, dtype=np.float32)
    b, s, d = x.shape
    assert (b, s, d) == (B, S, D)

    xT_np = np.ascontiguousarray(x.reshape(BS, D).T).astype(np.float16)
    ident_np = np.eye(128, dtype=np.float16)

    def sbuf_layout(wT, m):
        # [D, m] -> [128, (D//128)*m]: partition-major chunks, contiguous rows
        return np.ascontiguousarray(
            wT.reshape(NDC, 128, m).transpose(1, 0, 2).reshape(128, NDC * m))

    woT_np = sbuf_layout(np.ascontiguousarray(
        output_proj.astype(np.float32).T).astype(np.float16), D)

    in_maps = []
    for c in range(NCORES):
        h0 = HPC * c
        wq = sbuf_layout(np.ascontiguousarray(
            q_heads[h0:h0 + HPC].astype(np.float32).reshape(HPC * DK, D).T
        ).astype(np.float16), 128)
        wk = sbuf_layout(np.ascontiguousarray(
            k_heads[h0:h0 + HPC].astype(np.float32).reshape(HPC * DK, D).T
        ).astype(np.float16), 128)
        wv = sbuf_layout(np.ascontiguousarray(
            v_heads[h0:h0 + HPC].astype(np.float32).reshape(HPC * DK, D).T
        ).astype(np.float16), 128)
        in_maps.append({
            "xT": xT_np, "wqT": wq, "wkT": wk, "wvT": wv,
            "woT": woT_np, "ident": ident_np,
            "tri": np.triu(np.ones((128, 128), dtype=np.float16)),
        })
    return in_maps


def assemble(results):
    finalT = np.empty((D, BS), dtype=np.float32)
    for c in range(NCORES):
        for b in range(B):
            finalT[:, b * S + c * HS:b * S + (c + 1) * HS] = \
                results[c]["out"][:, b * HS:(b + 1) * HS].astype(np.float32)
    return np.ascontiguousarray(finalT.T).reshape(B, S, D)


def kernel(x, q_heads, k_heads, v_heads, output_proj):
    in_maps = prepare_in_maps(x, q_heads, k_heads, v_heads, output_proj)
    if "nc" not in _COMPILED:
        _COMPILED["nc"] = build_program()
    res = run_bass_kernel_spmd(_COMPILED["nc"], in_maps, list(range(NCORES)))
    return assemble(res.results)


# revision 29
# speedup vs baseline: 1.0543x; 1.0543x over previous
"""Multi-head causal self-attention on 8 Trainium2 NeuronCores (Bass/Tile).

Sharding: heads 2c,2c+1 -> core c (both batches). Each core computes its two
heads' attention output (concat^T rows [128c, 128c+128)), per-batch AllToAlls
redistribute so core c holds concat^T[:, 256c:256c+256] of each batch, and
each core runs the output projection for its two 256-col s-slices over full d.
Host assembles the 8 [1024, 512] slices and transposes back to [2, 2048, 1024].

All matmuls run in fp16 (tolerance 2e-2; measured rel err ~7e-4).

Schedule highlights:
- a tiny dummy AllToAll fires at program start so the collective stack's
  first-use rendezvous cost (~11us trigger delay) is paid off the critical
  path; the real A2As then trigger with ~1us delay;
- one AllToAll per batch; A2A0 triggers right after batch 0's epilogue and
  completes under batch 1's attention;
- exp fused across both heads per chunk via one 3-D strided activation
  (ScalarE is ~95% busy during attention - instruction count matters);
- causal masks on gpsimd for BOTH batches (DVE freed for evict/norm work);
  the cb0 gather runs on the gpsimd SWDGE after the last b1 mask, so it
  never blocks mask progress, and the A2A1 trigger follows it;
- cb1 gather on the sync HWDGE queue after out-proj(b0)'s stores (a DMA
  trigger stalls its issuing engine while waiting, so order matters);
- x tiles stream on both HWDGE queues for the whole projection phase
  (one queue saturates at ~160 GB/s; two cut the phase's DMA stalls);
  wo loads after all x traffic since it's needed only at the end;
- weights host-prearranged to SBUF layout; V evicts split per 128-col
  quarter; output stored fp16 per-chunk; softmax-denom broadcast via cheap
  K=1 fp16 ones-matmuls; PV pipeline depth 2; exp trimmed to valid columns.
"""
import sys

sys.path.insert(0, "/opt/trn_rl_repo")

import numpy as np
import concourse.bacc as bacc
import concourse.mybir as mybir
from concourse.tile import TileContext
from concourse.bass_utils import run_bass_kernel_spmd

B = 2
S = 2048
D = 1024
H = 16
DK = 64
NCORES = 8
HPC = H // NCORES          # heads per core = 2
BS = B * S                 # 4096
NDC = D // 128             # 8 d-chunks
QB = 512                   # q-block width
NQB = S // QB              # 4 q-blocks per batch
NKC = S // 128             # 16 kpos chunks per batch
HS = 256                   # per-core s-slice per batch (A2A shard width)

F32 = mybir.dt.float32
F16 = mybir.dt.float16
AF = mybir.ActivationFunctionType

_COMPILED = {}


def build_program():
    nc = bacc.Bacc(None, target_bir_lowering=False, debug=False)

    # ---- DRAM I/O (per-core tensors; host supplies per-core slices) ----
    xT = nc.dram_tensor("xT", [D, BS], F16, kind="ExternalInput")
    # weights host-prearranged to the exact SBUF layout (contiguous DMAs,
    # no per-256B-descriptor rearrange storms)
    wqT = nc.dram_tensor("wqT", [128, NDC * 128], F16, kind="ExternalInput")
    wkT = nc.dram_tensor("wkT", [128, NDC * 128], F16, kind="ExternalInput")
    wvT = nc.dram_tensor("wvT", [128, NDC * 128], F16, kind="ExternalInput")
    woT = nc.dram_tensor("woT", [128, NDC * D], F16, kind="ExternalInput")
    ident = nc.dram_tensor("ident", [128, 128], F16, kind="ExternalInput")
    tri = nc.dram_tensor("tri", [128, 128], F16, kind="ExternalInput")
    out = nc.dram_tensor("out", [D, 2 * HS], F16, kind="ExternalOutput")
    outr = out.rearrange("(c p) m -> p c m", p=128)

    with TileContext(nc) as tc:
        with (
            tc.tile_pool(name="const", bufs=1) as const,
            tc.tile_pool(name="persist", bufs=1) as persist,
            tc.tile_pool(name="ct", bufs=1) as ctp,
            tc.tile_pool(name="dram", bufs=1, space="DRAM") as dram,
        ):
            a2a_in = [dram.tile([NCORES, 128, HS], F16, name=f"a2ain_{b}")
                      for b in range(B)]
            a2a_out = [dram.tile([NCORES, 128, HS], F16, name=f"a2aout_{b}")
                       for b in range(B)]
            warm_cc = [dram.tile([NCORES, 1, 16], F16, name=f"wcc{i}")
                       for i in range(2)]

            # projection weights first (the first matmul group needs wq);
            # each split across both HWDGE queues to halve load latency
            w_sb = {}
            for name in ("q", "k", "v"):
                w_sb[name] = const.tile([128, NDC * 128], F16, name=f"w{name}")
            nc.sync.dma_start(out=w_sb["q"][:, 0:4 * 128],
                              in_=wqT[:, 0:4 * 128])
            nc.scalar.dma_start(out=w_sb["q"][:, 4 * 128:],
                                in_=wqT[:, 4 * 128:])

            # dummy collective: pays the first-use rendezvous cost early so
            # the real A2As trigger with ~1us delay instead of ~11us
            nc.gpsimd.collective_compute(
                "AllToAll", mybir.AluOpType.bypass,
                replica_groups=[list(range(NCORES))],
                ins=[warm_cc[0].opt()], outs=[warm_cc[1].opt()])

            # small constants (no DMA where possible)
            id_sb = const.tile([128, 128], F16)
            nc.scalar.dma_start(out=id_sb[:], in_=ident[:])
            tri_sb = const.tile([128, 128], F16)
            nc.scalar.dma_start(out=tri_sb[:], in_=tri[:])
            nbias = const.tile([128, 1], F32)
            nc.vector.memset(nbias[:], -6.0)
            # ones row for broadcasting recip-sumexp to 64 partitions via a
            # cheap fp16 K=1 matmul
            ones16 = const.tile([1, 64], F16, name="ones16")
            nc.vector.memset(ones16[:], 1.0)

            # persistent activations
            QT = [persist.tile([128, S], F16, name=f"QT_{b}") for b in range(B)]
            KT = [persist.tile([128, S], F16, name=f"KT_{b}") for b in range(B)]
            # V tiles: [kpos 128, 130] = [Vh0 | 1 | Vh1 | 1] per (b, kpos-chunk)
            VP = [[persist.tile([128, 130], F16, name=f"VP_{b}_{sc}")
                   for sc in range(NKC)] for b in range(B)]
            for b in range(B):
                for sc in range(NKC):
                    nc.vector.memset(VP[b][sc][:, 64:65], 1.0)
                    nc.vector.memset(VP[b][sc][:, 129:130], 1.0)

            # gathered A2A results
            cb_tiles = [ctp.tile([128, NCORES * HS], F16, name=f"cb_{b}",
                                 tag=f"cb{b}") for b in range(B)]

            # ---- phase 1: projections (xt streamed in 512-col tiles) ----
            with (
                tc.tile_pool(name="xt", bufs=16) as xtp,
                tc.tile_pool(name="vt", bufs=2) as vtp,
                tc.tile_pool(name="pacc", bufs=4, space="PSUM") as pacc,
                tc.tile_pool(name="ptr", bufs=2, space="PSUM") as ptr,
            ):
                xt = {}
                for b in range(B):
                    for j in range(4):
                        for dc in range(NDC):
                            t = xtp.tile([128, QB], F16,
                                         name=f"xt_{b}_{j}_{dc}", tag="xt")
                            # both HWDGE queues stream x for the whole phase
                            eng = nc.sync if dc % 2 == 0 else nc.scalar
                            eng.dma_start(
                                out=t[:],
                                in_=xT[dc * 128:(dc + 1) * 128,
                                       b * S + j * QB:b * S + (j + 1) * QB])
                            xt[b, j, dc] = t
                        if b == 0 and j == 0:
                            nc.sync.dma_start(
                                out=w_sb["k"][:, 0:4 * 128],
                                in_=wkT[:, 0:4 * 128])
                            nc.scalar.dma_start(
                                out=w_sb["k"][:, 4 * 128:],
                                in_=wkT[:, 4 * 128:])
                            nc.sync.dma_start(
                                out=w_sb["v"][:, 0:4 * 128],
                                in_=wvT[:, 0:4 * 128])
                            nc.scalar.dma_start(
                                out=w_sb["v"][:, 4 * 128:],
                                in_=wvT[:, 4 * 128:])
                # out-proj weights after all x traffic: one 2MB transfer
                # that is only needed at ~150us
                wo_sb = const.tile([128, NDC * D], F16, name="wo")
                nc.scalar.dma_start(out=wo_sb[:], in_=woT[:])

                proj_scope = nc.named_scope("proj")
                proj_scope.__enter__()

                # warm-up matmuls on the identity tile: ~3us of PE activity so
                # the HAM clock gate un-throttles before the first projection
                # group (otherwise the first ~3.4us of real matmuls run at
                # 1.2GHz)
                warm0 = pacc.tile([128, 128], F32, name="warm0", tag="warm0",
                                  bufs=1)
                for _ in range(10):
                    nc.tensor.matmul(warm0[:], id_sb[:], id_sb[:],
                                     start=True, stop=True)

                for b in range(B):
                    for j in range(4):
                        for mat in ("q", "k", "v"):
                            ps = pacc.tile([128, QB], F32,
                                           name=f"ps_{mat}_{b}_{j}", tag="pacc")
                            for dc in range(NDC):
                                nc.tensor.matmul(
                                    ps[:],
                                    w_sb[mat][:, dc * 128:(dc + 1) * 128],
                                    xt[b, j, dc][:],
                                    start=(dc == 0), stop=(dc == NDC - 1))
                            if mat in ("q", "k"):
                                dst = QT[b] if mat == "q" else KT[b]
                                nc.scalar.copy(dst[:, j * QB:(j + 1) * QB], ps[:])
                            else:
                                vt = vtp.tile([128, QB], F16,
                                              name=f"vt_{b}_{j}", tag="vt")
                                for t4 in range(4):
                                    # evict per 128-col quarter so each
                                    # transpose starts as soon as its slice
                                    # lands instead of after the full copy
                                    nc.scalar.copy(
                                        vt[:, t4 * 128:(t4 + 1) * 128],
                                        ps[:, t4 * 128:(t4 + 1) * 128])
                                for t4 in range(4):
                                    sc = 4 * j + t4
                                    tr = ptr.tile([128, 128], F16,
                                                  name=f"tr_{b}_{sc}", tag="tr")
                                    nc.tensor.transpose(
                                        tr[:], vt[:, t4 * 128:(t4 + 1) * 128],
                                        id_sb[:])
                                    nc.vector.tensor_copy(
                                        VP[b][sc][:, 0:64], tr[:, 0:64])
                                    nc.vector.tensor_copy(
                                        VP[b][sc][:, 65:129], tr[:, 64:128])

            proj_scope.__exit__(None, None, None)
            # ---- phase 2: attention + per-batch A2A ----
            with (
                tc.tile_pool(name="pp", bufs=8) as pp,
                tc.tile_pool(name="osb", bufs=2) as osb,
            ):
                attn_psum = [
                    tc.tile_pool(name="psc", bufs=2, space="PSUM"),
                    tc.tile_pool(name="po", bufs=3, space="PSUM"),
                    tc.tile_pool(name="pbc", bufs=1, space="PSUM"),
                ]
                psc, po, pbc = [p.__enter__() for p in attn_psum]

                attn_scope = nc.named_scope("attn")
                attn_scope.__enter__()

                # Flat chunk stream with a cross-block 2-deep PV pipeline.
                # All of b0 first: A2A0 dispatches at mid-attention and its
                # completion hides entirely under batch 1's compute.
                order = [(0, 0), (0, 1), (0, 2), (0, 3),
                         "A2A0", (1, 0), (1, 1), (1, 2), (1, 3), "A2A1"]

                class Blk:
                    pass

                stream = []
                for item in order:
                    if isinstance(item, str):
                        stream.append(item)
                        continue
                    b, jb = item
                    blk = Blk()
                    blk.b, blk.jb = b, jb
                    blk.nchunk = 4 * jb + 4
                    stream.extend((blk, i) for i in range(blk.nchunk))

                def emit_a2a(b):
                    nc.gpsimd.collective_compute(
                        "AllToAll",
                        mybir.AluOpType.bypass,
                        replica_groups=[list(range(NCORES))],
                        ins=[a2a_in[b].opt()],
                        outs=[a2a_out[b].opt()],
                    )

                def emit_pv(blk, i, v0):
                    for h in range(HPC):
                        nc.tensor.matmul(
                            blk.o_ps[h][:, v0:QB],
                            VP[blk.b][i][:, 65 * h:65 * h + 65],
                            blk.pblk[i][:, h:h + 1, v0:QB],
                            start=(i == 0), stop=(i == blk.nchunk - 1))
                    if i == blk.nchunk - 1:
                        emit_evict(blk)

                def emit_evict(blk):
                    # evict accumulators (frees po PSUM) + fast reciprocal of
                    # both heads' sumexp rows (read straight from PSUM)
                    b, jb = blk.b, blk.jb
                    blk.ou = [osb.tile([65, QB], F32, name=f"ou_{b}_{jb}_{h}",
                                       tag="ou") for h in range(HPC)]
                    se = osb.tile([1, 2 * QB], F32, name=f"se_{b}_{jb}",
                                  tag="se")
                    for h in range(HPC):
                        nc.vector.tensor_copy(blk.ou[h][:], blk.o_ps[h][:])
                        nc.vector.tensor_copy(
                            se[0:1, h * QB:(h + 1) * QB],
                            blk.o_ps[h][64:65, :])
                    rc = osb.tile([1, 2 * QB], F32, name=f"rx_{b}_{jb}",
                                  tag="rc2")
                    nc.vector.reciprocal_approx_fast(rc[:], se[:])
                    rr = osb.tile([1, 2 * QB], F16, name=f"rr_{b}_{jb}",
                                  tag="recr")
                    nc.vector.tensor_copy(rr[:], rc[:])
                    blk.rec_r = rr
                    pending_norm.append(blk)

                def flush_norm():
                    while pending_norm:
                        blk = pending_norm.pop(0)
                        b, jb = blk.b, blk.jb
                        o2 = osb.tile([128, QB], F16, name=f"o2_{b}_{jb}",
                                      tag="o2")
                        bc = pbc.tile([128, QB], F32, name=f"bc_{b}_{jb}",
                                      tag="bc")
                        for h in range(HPC):
                            nc.tensor.matmul(
                                bc[h * 64:(h + 1) * 64, :], ones16[:],
                                blk.rec_r[0:1, h * QB:(h + 1) * QB],
                                start=True, stop=True)
                        for h in range(HPC):
                            nc.vector.tensor_tensor(
                                out=o2[h * 64:(h + 1) * 64, :],
                                in0=blk.ou[h][0:64, :],
                                in1=bc[h * 64:(h + 1) * 64, :],
                                op=mybir.AluOpType.mult)
                        for half in range(2):
                            nc.sync.dma_start(
                                out=a2a_in[b][2 * jb + half, :, :],
                                in_=o2[:, half * HS:(half + 1) * HS])

                pending_norm = []
                pvq = []           # [(blk, i, v0)] awaiting emission (depth 2)
                nchunks_seen = 0
                for item in stream:
                    if isinstance(item, str):
                        # flush pipeline + norms before the collective so its
                        # input DMAs are emitted
                        while pvq:
                            emit_pv(*pvq.pop(0))
                        flush_norm()
                        if item == "A2A0":
                            emit_a2a(0)
                        else:
                            # A2A1 trigger first (gpsimd has nothing else
                            # pending; it fires the moment b1's epilogue
                            # stores land), then the cb0 gather
                            emit_a2a(1)
                            nc.gpsimd.dma_start(
                                out=cb_tiles[0][:],
                                in_=a2a_out[0].rearrange("r p m -> p r m"))
                        continue
                    blk, i = item
                    b, jb = blk.b, blk.jb
                    if i == 0:
                        blk.o_ps = [po.tile([65, QB], F32,
                                            name=f"o_{b}_{jb}_{h}", tag="po")
                                    for h in range(HPC)]
                        blk.pblk = {}
                    r = i - 4 * jb
                    v0 = max(0, r) * 128   # first valid q col in block
                    s_ps = psc.tile([128, HPC, QB], F32,
                                    name=f"s_{b}_{jb}_{i}", tag="psc")
                    for h in range(HPC):
                        nc.tensor.matmul(
                            s_ps[:, h:h + 1, v0:QB],
                            KT[b][h * 64:(h + 1) * 64, i * 128:(i + 1) * 128],
                            QT[b][h * 64:(h + 1) * 64,
                                  jb * QB + v0:(jb + 1) * QB],
                            start=True, stop=True)
                    nchunks_seen += 1
                    if nchunks_seen % 4 == 2:
                        flush_norm()
                    p_sb = pp.tile([128, HPC, QB], F16,
                                   name=f"p_{b}_{jb}_{i}", tag="pp")
                    blk.pblk[i] = p_sb
                    # bias -6 keeps exp in fp16 range (max raw score*scale is
                    # ~12); normalization cancels it. One strided activation
                    # covers both heads.
                    nc.scalar.activation(p_sb[:, :, v0:QB], s_ps[:, :, v0:QB],
                                         AF.Exp, scale=0.125, bias=nbias[:])
                    if r >= 0:
                        # zero the partial 128-col diagonal block: gpsimd in
                        # b0 (idle), DVE in b1 so the gpsimd stream is free
                        # to run the A2A1 trigger the moment b1's epilogue
                        # stores land (not hostage to mask progress)
                        eng = nc.gpsimd if b == 0 else nc.vector
                        for h in range(HPC):
                            sl = p_sb[:, h:h + 1, v0:v0 + 128]
                            eng.tensor_tensor(
                                out=sl, in0=sl, in1=tri_sb[:],
                                op=mybir.AluOpType.mult)
                    while len(pvq) >= 2:
                        emit_pv(*pvq.pop(0))
                    pvq.append((blk, i, v0))
                while pvq:
                    emit_pv(*pvq.pop(0))
                attn_scope.__exit__(None, None, None)
                for p in reversed(attn_psum):
                    p.__exit__(None, None, None)

                # out-proj(b0) runs as soon as the PE drains attention (cb0
                # gathered under attn-b1's tail); out-proj(b1) after the cb1
                # gather, which queues on sync BEHIND out-proj(b0)'s stores.
                # PSUM pool opened after the attn pools close so the bank
                # allocator reuses their banks.
                with tc.tile_pool(name="pf", bufs=2, space="PSUM") as pf:
                    def warm_fill(n):
                        # dependency-free matmuls bridging residual collective
                        # wait so the HAM clock gate doesn't re-throttle
                        wp = pf.tile([128, QB], F32, name="warm_ps",
                                     tag="warm", bufs=1)
                        for _ in range(n):
                            nc.tensor.matmul(wp[:], id_sb[:], KT[0][:, 0:QB],
                                             start=True, stop=True)

                    def outproj(b):
                        for ds in range(NDC):
                            f_ps = pf.tile([128, HS], F32, name=f"f_{b}_{ds}",
                                           tag="pf")
                            for kc in range(NDC):
                                nc.tensor.matmul(
                                    f_ps[:],
                                    wo_sb[:, kc * D + ds * 128:
                                          kc * D + (ds + 1) * 128],
                                    cb_tiles[b][:, kc * HS:(kc + 1) * HS],
                                    start=(kc == 0), stop=(kc == NDC - 1))
                            fb = osb.tile([128, HS], F16, name=f"fb_{b}_{ds}",
                                          tag="fb")
                            nc.vector.tensor_copy(fb[:], f_ps[:])
                            nc.sync.dma_start(
                                out=outr[:, ds, b * HS:(b + 1) * HS],
                                in_=fb[:])

                    warm_fill(8)
                    outproj(0)
                    # cb1 gather on sync AFTER b0's out stores (trigger
                    # stalls the queue head while waiting on the collective)
                    nc.sync.dma_start(
                        out=cb_tiles[1][:],
                        in_=a2a_out[1].rearrange("r p m -> p r m"))
                    warm_fill(4)
                    outproj(1)

    nc.finalize()
    return nc


def prepare_in_maps(x, q_heads, k_heads, v_heads, output_proj):
    x = np.ascontiguousarray(x, dtype=np.float32)
    b, s, d = x.shape
    assert (b, s, d) == (B, S, D)

    xT_np = np.ascontiguousarray(x.reshape(BS, D).T).astype(np.float16)
    ident_np = np.eye(128, dtype=np.float16)

    def sbuf_layout(wT, m):
        # [D, m] -> [128, (D//128)*m]: partition-major chunks, contiguous rows
        return np.ascontiguousarray(
            wT.reshape(NDC, 128, m).transpose(1, 0, 2).reshape(128, NDC * m))

    woT_np = sbuf_layout(np.ascontiguousarray(
        output_proj.astype(np.float32).T).astype(np.float16), D)

    in_maps = []
    for c in range(NCORES):
        h0 = HPC * c
        wq = sbuf_layout(np.ascontiguousarray(
            q_heads[h0:h0 + HPC].astype(np.float32).reshape(HPC * DK, D).T
        ).astype(np.float16), 128)
        wk = sbuf_layout(np.ascontiguousarray(
            k_heads[h0:h0 + HPC].astype(np.float32).reshape(HPC * DK, D).T
        ).astype(np.float16), 128)
        wv = sbuf_layout(np.ascontiguousarray(
            v_heads[h0:h0 + HPC].astype(np.float32).reshape(HPC * DK, D).T
        ).astype(np.float16), 128)
        in_maps.append({
            "xT": xT_np, "wqT": wq, "wkT": wk, "wvT": wv,
            "woT": woT_np, "ident": ident_np,
            "tri": np.triu(np.ones((128, 128), dtype=np.float16)),
        })
    return in_maps


def assemble(results):
    finalT = np.empty((D, BS), dtype=np.float32)
    for c in range(NCORES):
        for b in range(B):
            finalT[:, b * S + c * HS:b * S + (c + 1) * HS] = \
                results[c]["out"][:, b * HS:(b + 1) * HS].astype(np.float32)
    return np.ascontiguousarray(finalT.T).reshape(B, S, D)


def kernel(x, q_heads, k_heads, v_heads, output_proj):
    in_maps = prepare_in_maps(x, q_heads, k_heads, v_heads, output_proj)
    if "nc" not in _COMPILED:
        _COMPILED["nc"] = build_program()
    res = run_bass_kernel_spmd(_COMPILED["nc"], in_maps, list(range(NCORES)))
    return assemble(res.results)
